# revision 17
# baseline (speedup 1.0000x reference)
"""GPT (4-layer, E=768, H=12, T=1024, B=2, V=50257) forward on 8 trn2 cores.

Sharding (v4):
  - Token-parallel residual: core c owns tokens [r*256,(r+1)*256) of batch
    c//4 (r = c%4).  LN / QKV / out-proj / MLP fully token-local.
  - LN affine (gamma/beta) folded into the following matmul weights on the
    host (exact), so device LN is a pure normalize.
  - Attention: per layer ONE AllGather per batch group of 4 carrying both
    K (dim-major [768,256]) and V (token-major [256,780], softmax ones
    column pre-packed) in a single packed bf16 buffer.  Each core computes
    all 12 heads for its own 256 queries against the full 1024 keys with a
    per-core causal mask (masking by data, not control flow).
  - Softmax normalize: denominators from the V ones-column; fast approx
    reciprocal + PE K=1 broadcast matmul + one DVE multiply per head pair.
  - lm_head vocab-sharded with wte STATIONARY (1 LDWEIGHTS per 4 N=512
    matmuls): each core computes a [6400, 2048] logit slice (V padded to
    51200), host transposes/concats.
  - All matmul operands bf16 (fp32 PSUM accumulation).
"""

import sys
from contextlib import ExitStack
import numpy as np
import ml_dtypes

sys.path.insert(0, "/opt/trn_rl_repo")

import concourse.bass as bass
import concourse.mybir as mybir
import concourse.tile as tile
from concourse import bacc
from concourse.bass_utils import run_bass_kernel_spmd
from concourse.masks import make_identity

L, H, E, T, V = 4, 12, 768, 1024, 50257
B = 2
NC = 8
TS = (B * T) // NC          # 256 tokens per core
VS = 6400                   # vocab slice per core (padded V = 51200)
VPAD = VS * NC
HD = 64
EPS = 1e-5
SCALE = float(1.0 / np.sqrt(np.float32(E)))
F32 = mybir.dt.float32
F32R = mybir.dt.float32r
BF16 = mybir.dt.bfloat16
BF = ml_dtypes.bfloat16

KB_EL = E * TS              # K elems in packed kv buffer
VB_EL = TS * H * 65         # V elems
KVN = KB_EL + VB_EL

_CACHE = {}


def _build_program():
    nc = bacc.Bacc("TRN2", target_bir_lowering=False, debug=False, num_devices=NC)

    # ---- I/O -------------------------------------------------------------
    x0s = nc.dram_tensor("x0s", [TS, E], F32, kind="ExternalInput")
    wq = nc.dram_tensor("wq", [L, E, E], BF16, kind="ExternalInput")
    bq = nc.dram_tensor("bq", [L, H, HD], F32, kind="ExternalInput")
    wk = nc.dram_tensor("wk", [L, E, E], BF16, kind="ExternalInput")
    bk = nc.dram_tensor("bk", [L, 6, 128], F32, kind="ExternalInput")
    wv = nc.dram_tensor("wv", [L, E, E], BF16, kind="ExternalInput")
    bv = nc.dram_tensor("bv", [L, E], BF16, kind="ExternalInput")
    watp = nc.dram_tensor("watp", [L, H, HD, E], BF16, kind="ExternalInput")
    atpb = nc.dram_tensor("atpb", [L, E], BF16, kind="ExternalInput")
    fcw = nc.dram_tensor("fcw", [L, 6, 128, 4 * E], BF16, kind="ExternalInput")
    fcb = nc.dram_tensor("fcb", [L, 24, 128], F32, kind="ExternalInput")
    prw = nc.dram_tensor("prw", [L, 4 * E, E], BF16, kind="ExternalInput")
    prb = nc.dram_tensor("prb", [L, E], BF16, kind="ExternalInput")
    mask = nc.dram_tensor("mask", [8, 128, 2 * TS], BF16, kind="ExternalInput")
    wteT = nc.dram_tensor("wteT", [E, VS], BF16, kind="ExternalInput")
    logits = nc.dram_tensor("logits", [VS, B * T], F32, kind="ExternalOutput")

    g_all = [list(range(NC))]
    g_batch = [[0, 1, 2, 3], [4, 5, 6, 7]]

    def bcast_row(pool, src_ap, n, dtype, w, name="bc"):
        """Replicate a [w] DRAM row across n partitions via broadcast DMA."""
        t = pool.tile([n, w], dtype, tag=name, name=name)
        in_ap = bass.AP(
            tensor=src_ap.tensor,
            offset=src_ap.offset,
            ap=[[0, n]] + [list(p) for p in src_ap.ap],
        )
        nc.sync.dma_start(out=t[:], in_=in_ap)
        return t

    with tile.TileContext(nc) as tc, ExitStack() as es:
        const = es.enter_context(tc.tile_pool(name="const", bufs=1))
        xp = es.enter_context(tc.tile_pool(name="xp", bufs=1))
        lnrow = es.enter_context(tc.tile_pool(name="lnrow", bufs=2))
        stat = es.enter_context(tc.tile_pool(name="stat", bufs=4))
        hpool = es.enter_context(tc.tile_pool(name="hpool", bufs=2))
        hTp = es.enter_context(tc.tile_pool(name="hTp", bufs=1))
        dram = es.enter_context(tc.tile_pool(name="dram", bufs=1, space="DRAM"))

        # warm up the collective rings while the first layer computes
        warm_in = dram.tile([256], BF16)
        warm_out1 = dram.tile([1024], BF16)
        warm_out2 = dram.tile([2048], BF16)
        nc.gpsimd.collective_compute(
            "AllGather", mybir.AluOpType.bypass, replica_groups=g_batch,
            ins=[warm_in[:]], outs=[warm_out1[:]])
        nc.gpsimd.collective_compute(
            "AllGather", mybir.AluOpType.bypass, replica_groups=g_all,
            ins=[warm_in[:]], outs=[warm_out2[:]])

        # persistent residual stream [256, 768] as two [128, 768] f32 tiles
        x_sb = [xp.tile([128, E], F32, tag=f"x{t}", name=f"x{t}") for t in range(2)]
        for t in range(2):
            nc.sync.dma_start(out=x_sb[t][:], in_=x0s[t * 128:(t + 1) * 128, :])

        ident_f = const.tile([128, 128], F32, name="ident_f")
        make_identity(nc, ident_f)
        ident = const.tile([128, 128], F32R, name="ident")
        nc.vector.tensor_copy(ident[:], ident_f[:])
        eps_sb = const.tile([128, 1], F32)
        nc.vector.memset(eps_sb, EPS)
        ones780 = const.tile([128, H * 65], BF16, name="ones780")
        nc.vector.memset(ones780, 1.0)
        mask_sb = [const.tile([128, 2 * TS], BF16, name=f"mask{j}") for j in range(8)]

        # DRAM bounce buffers for collectives (K and V packed in one buffer)
        kv_in = dram.tile([KVN], BF16)
        kv_ag = dram.tile([4 * KVN], BF16)
        xf_in = dram.tile([E, TS], BF16)
        xf_ag = dram.tile([NC * E, TS], BF16, addr_space="Shared")
        kv_in_b = kv_in[:]
        kv_ag_b = kv_ag[:]

        def layernorm2(x_tiles, out_tiles):
            """Pure normalize (no affine) of two [128,768] f32 tiles."""
            stats = stat.tile([128, 2, 3, 6], F32, tag="bn_stats", name="bn_stats_t")
            for t in range(2):
                xr = x_tiles[t][:].rearrange("p (s d) -> p s d", s=3)
                for s in range(3):
                    nc.vector.bn_stats(out=stats[:, t, s, :], in_=xr[:, s, :])
            mv = stat.tile([128, 2, 2], F32, tag="bn_aggr", name="bn_aggr_t")
            for t in range(2):
                nc.vector.bn_aggr(out=mv[:, t, :], in_=stats[:, t])
            std = stat.tile([128, 2], F32, tag="std", name="std_t")
            for t in range(2):
                nc.scalar.activation(out=std[:, t:t + 1], in_=mv[:, t, 1:2],
                                     func=mybir.ActivationFunctionType.Sqrt,
                                     bias=eps_sb[:], scale=1.0)
            rstd = stat.tile([128, 2], F32, tag="rstd", name="rstd_t")
            nc.vector.reciprocal_approx_fast(out=rstd[:], in_=std[:])
            for t in range(2):
                nc.vector.tensor_scalar(out=out_tiles[t][:], in0=x_tiles[t][:],
                                        scalar1=mv[:, t, 0:1],
                                        scalar2=rstd[:, t:t + 1],
                                        op0=mybir.AluOpType.subtract,
                                        op1=mybir.AluOpType.mult)

        def scope(name):
            sid, _ = nc.enter_named_scope(name, False)
            _SCOPES.append((name, sid))

        def unscope():
            name, sid = _SCOPES.pop()
            nc.leave_named_scope(name, sid, False)

        _SCOPES = []

        es_l = es.enter_context(ExitStack())
        wqp = es_l.enter_context(tc.tile_pool(name="wqp", bufs=1))
        wkp = es_l.enter_context(tc.tile_pool(name="wkp", bufs=1))
        wvp = es_l.enter_context(tc.tile_pool(name="wvp", bufs=1))
        watpp = es_l.enter_context(tc.tile_pool(name="watpp", bufs=1))
        bias_p = es_l.enter_context(tc.tile_pool(name="bias_p", bufs=2))
        qTp = es_l.enter_context(tc.tile_pool(name="qTp", bufs=1))
        kvsp = es_l.enter_context(tc.tile_pool(name="kvsp", bufs=2))
        kTp = es_l.enter_context(tc.tile_pool(name="kTp", bufs=1))
        vp = es_l.enter_context(tc.tile_pool(name="vp", bufs=1))
        ep = es_l.enter_context(tc.tile_pool(name="ep", bufs=3))
        yp = es_l.enter_context(tc.tile_pool(name="yp", bufs=1))
        sm = es_l.enter_context(tc.tile_pool(name="sm", bufs=2))
        fcwp = es_l.enter_context(tc.tile_pool(name="fcwp", bufs=1))
        mTp = es_l.enter_context(tc.tile_pool(name="mTp", bufs=1))
        prwp = es_l.enter_context(tc.tile_pool(name="prwp", bufs=5))
        psT = es_l.enter_context(tc.tile_pool(name="psT", bufs=2, space="PSUM"))

        def transpose_to(hsrc, dst_tiles, dst_col):
            """hsrc [128,768] f32r -> bf16 dst_tiles[k][:, dst_col:dst_col+128]."""
            for k in range(6):
                pt = psT.tile([128, 128], F32R, tag="tr", name="tr")
                nc.tensor.transpose(pt[:], hsrc[:, k * 128:(k + 1) * 128], ident[:])
                dst = dst_tiles[k][:, dst_col:dst_col + 128]
                if k % 2 == 0:
                    nc.vector.tensor_copy(dst, pt[:].bitcast(F32))
                else:
                    nc.scalar.activation(out=dst, in_=pt[:].bitcast(F32),
                                         func=mybir.ActivationFunctionType.Copy)

        def load_layer_weights(layer):
            w = {}
            w["wk"] = [wkp.tile([128, E], BF16, tag=f"wk{k}", name=f"wk{k}")
                       for k in range(6)]
            for k in range(6):
                nc.sync.dma_start(out=w["wk"][k][:],
                                  in_=wk[layer, k * 128:(k + 1) * 128, :])
            w["bk"] = bias_p.tile([128, 6], F32, tag="bk", name="bk")
            nc.sync.dma_start(out=w["bk"][:], in_=bk[layer].transpose([1, 0]))
            w["wv"] = [wvp.tile([128, E], BF16, tag=f"wv{k}", name=f"wv{k}")
                       for k in range(6)]
            for k in range(6):
                nc.sync.dma_start(out=w["wv"][k][:],
                                  in_=wv[layer, k * 128:(k + 1) * 128, :])
            w["wq"] = [wqp.tile([128, E], BF16, tag=f"wq{k}", name=f"wq{k}")
                       for k in range(6)]
            for k in range(6):
                nc.sync.dma_start(out=w["wq"][k][:],
                                  in_=wq[layer, k * 128:(k + 1) * 128, :])
            w["bq"] = bias_p.tile([64, H], F32, tag="bq", name="bq")
            nc.sync.dma_start(out=w["bq"][:], in_=bq[layer].transpose([1, 0]))
            w["watp"] = [watpp.tile([64, E], BF16, tag=f"watp{h}", name=f"watp{h}")
                         for h in range(H)]
            for h in range(H):
                nc.sync.dma_start(out=w["watp"][h][:], in_=watp[layer, h])
            w["fck"] = [fcwp.tile([128, 4 * E], BF16, tag=f"fck{k}", name=f"fck{k}")
                        for k in range(6)]
            for k in range(6):
                nc.sync.dma_start(out=w["fck"][k][:], in_=fcw[layer, k])
            w["fcb"] = bias_p.tile([128, 24], F32, tag="fcb", name="fcb")
            nc.sync.dma_start(out=w["fcb"][:], in_=fcb[layer].transpose([1, 0]))
            return w

        wcur = load_layer_weights(0)
        for j in range(8):
            nc.sync.dma_start(out=mask_sb[j][:], in_=mask[j])

        for layer in range(L):
            scope(f"L{layer}_qkv")
            wk_sb, wv_sb, wq_sb = wcur["wk"], wcur["wv"], wcur["wq"]
            bk_sb, bq_sb, fcb_sb = wcur["bk"], wcur["bq"], wcur["fcb"]
            watp_sb, fck = wcur["watp"], wcur["fck"]
            # ---- LN1 (pure normalize) + transpose -> hT [768, 256] bf16 ----
            hT = [hTp.tile([128, TS], BF16, tag=f"hT{k}", name=f"hT{k}") for k in range(6)]
            h_t = [hpool.tile([128, E], F32R, tag="h", name="h") for t in range(2)]
            layernorm2(x_sb, h_t)
            for t in range(2):
                transpose_to(h_t[t], hT, t * 128)

            bv_bc = bcast_row(lnrow, bv[layer], 128, BF16, E, "bv")

            es_a = ExitStack()
            psK = es_a.enter_context(tc.tile_pool(name="psK", bufs=2, space="PSUM"))
            psV = es_a.enter_context(tc.tile_pool(name="psV", bufs=1, space="PSUM"))
            for m in range(6):
                ps = psK.tile([128, TS], F32, tag="kps", name="k_ps")
                for k in range(6):
                    nc.tensor.matmul(ps[:], wk_sb[k][:, m * 128:(m + 1) * 128],
                                     hT[k][:], start=(k == 0), stop=(k == 5))
                kt = kvsp.tile([128, TS], BF16, tag="kloc", name="kloc")
                nc.vector.tensor_scalar_add(out=kt[:], in0=ps[:],
                                            scalar1=bk_sb[:, m:m + 1])
                nc.sync.dma_start(
                    out=bass.AP(tensor=kv_in_b.tensor,
                                offset=kv_in_b.offset + m * 128 * TS,
                                ap=[[TS, 128], [1, TS]]),
                    in_=kt[:])

            # ---- V (token-major, ones column pre-packed) ----
            for t in range(2):
                ps = psV.tile([128, E], F32, tag="vps", name="v_ps")
                for n0, n1 in ((0, 512), (512, 768)):
                    for k in range(6):
                        nc.tensor.matmul(ps[:, n0:n1],
                                         hT[k][:, t * 128:(t + 1) * 128],
                                         wv_sb[k][:, n0:n1],
                                         start=(k == 0), stop=(k == 5))
                vt = kvsp.tile([128, H * 65], BF16, tag="vloc", name="vloc")
                nc.vector.tensor_copy(vt[:], ones780[:])
                for h in range(H):
                    nc.vector.tensor_add(out=vt[:, 65 * h:65 * h + 64],
                                         in0=ps[:, 64 * h:64 * h + 64],
                                         in1=bv_bc[:, 64 * h:64 * h + 64])
                nc.sync.dma_start(
                    out=bass.AP(tensor=kv_in_b.tensor,
                                offset=kv_in_b.offset + KB_EL + t * 128 * (H * 65),
                                ap=[[H * 65, 128], [1, H * 65]]),
                    in_=vt[:])
            nc.gpsimd.collective_compute(
                "AllGather", mybir.AluOpType.bypass,
                replica_groups=g_batch,
                ins=[kv_in[:]],
                outs=[kv_ag[:]],
            )

            # ---- Q per head (overlaps the AllGather) ----
            psQ = es_a.enter_context(tc.tile_pool(name="psQ", bufs=2, space="PSUM"))
            qT = [qTp.tile([64, TS], BF16, tag=f"qT{h}", name=f"qT{h}") for h in range(H)]
            for h in range(H):
                ps = psQ.tile([64, TS], F32, tag="q", name="q_ps")
                for k in range(6):
                    nc.tensor.matmul(ps[:], wq_sb[k][:, h * 64:(h + 1) * 64],
                                     hT[k][:], start=(k == 0), stop=(k == 5))
                nc.vector.tensor_scalar_add(out=qT[h][:], in0=ps[:],
                                            scalar1=bq_sb[:, h:h + 1])
            es_a.close()

            # prefetch next layer's weights into the AllGather/attention window
            wnext = load_layer_weights(layer + 1) if layer + 1 < L else None

            unscope()
            scope(f"L{layer}_attn")
            # ---- load gathered K (per-head) and V from the packed buffer ----
            kTh = [kTp.tile([64, T], BF16, tag=f"kTh{h}", name=f"kTh{h}")
                   for h in range(H)]
            v_sb = [vp.tile([128, H * 65], BF16, tag=f"v{j}", name=f"v{j}")
                    for j in range(8)]

            def load_kth(h):
                nc.sync.dma_start(
                    out=kTh[h][:].rearrange("p (r c) -> p r c", r=4),
                    in_=bass.AP(tensor=kv_ag_b.tensor,
                                offset=kv_ag_b.offset + h * 64 * TS,
                                ap=[[TS, 64], [KVN, 4], [1, TS]]))

            def load_v(j):
                nc.sync.dma_start(
                    out=v_sb[j][:],
                    in_=bass.AP(tensor=kv_ag_b.tensor,
                                offset=(kv_ag_b.offset + (j // 2) * KVN + KB_EL
                                        + (j % 2) * 128 * (H * 65)),
                                ap=[[H * 65, 128], [1, H * 65]]))

            load_kth(0)
            load_kth(1)
            for j in range(8):
                load_v(j)
            for h in range(2, H):
                load_kth(h)

            # ---- attention: 12 heads, 256 queries vs 1024 masked keys ----
            es_b = ExitStack()
            psS = es_b.enter_context(tc.tile_pool(name="psS", bufs=2, space="PSUM"))
            psY = es_b.enter_context(tc.tile_pool(name="psY", bufs=2, space="PSUM"))
            yT2 = [yp.tile([64, 2 * TS], BF16, tag=f"yT{hp}", name=f"yT{hp}")
                   for hp in range(6)]

            def softmax_epilogue(hp, yps):
                # den rows -> approx recip -> PE broadcast -> mul
                den = sm.tile([1, 2 * TS], F32, tag="den", name="den")
                for i in range(2):
                    nc.vector.tensor_copy(den[:, i * TS:(i + 1) * TS],
                                          yps[i][64:65, :])
                rec = sm.tile([1, 2 * TS], F32, tag="rec", name="rec")
                nc.vector.reciprocal_approx_fast(out=rec[:], in_=den[:])
                rec_bf = sm.tile([1, 2 * TS], BF16, tag="rec_bf", name="rec_bf")
                nc.vector.tensor_copy(rec_bf[:], rec[:])
                psb = psT.tile([64, 2 * TS], F32, tag="tr", name="bc_ps")
                nc.tensor.matmul(psb[:], ones780[0:1, 0:64], rec_bf[:],
                                 start=True, stop=True)
                rbc = sm.tile([64, 2 * TS], BF16, tag="rbc", name="rbc")
                nc.vector.tensor_copy(rbc[:], psb[:])
                for i in range(2):
                    nc.vector.tensor_mul(out=yT2[hp][:, i * TS:(i + 1) * TS],
                                         in0=yps[i][0:64, :],
                                         in1=rbc[:, i * TS:(i + 1) * TS])

            # software-pipelined: scores for j+1 issue before AV for j, and
            # the previous head-pair's epilogue issues after this pair's
            # first scores, so the PE never waits on the exp/mask chain.
            pending = None
            for hp in range(6):
                yps = [psY.tile([65, TS], F32, tag=f"y{i}", name=f"y{i}")
                       for i in range(2)]
                e_hold = {}
                for j in range(8):
                    pss = psS.tile([128, 2 * TS], F32, tag="s", name="s_ps")
                    e_sb = ep.tile([128, 2 * TS], BF16, tag="e", name="e")
                    for i in range(2):
                        nc.tensor.matmul(pss[:, i * TS:(i + 1) * TS],
                                         kTh[2 * hp + i][:, j * 128:(j + 1) * 128],
                                         qT[2 * hp + i][:], start=True, stop=True)
                    nc.scalar.activation(out=e_sb[:], in_=pss[:],
                                         func=mybir.ActivationFunctionType.Exp,
                                         scale=SCALE)
                    nc.vector.tensor_mul(out=e_sb[:], in0=e_sb[:], in1=mask_sb[j][:])
                    e_hold[j] = e_sb
                    if j == 1 and pending is not None:
                        softmax_epilogue(*pending)
                        pending = None
                    if j >= 1:
                        ej = e_hold.pop(j - 1)
                        for i in range(2):
                            h = 2 * hp + i
                            nc.tensor.matmul(yps[i][:],
                                             v_sb[j - 1][:, 65 * h:65 * h + 65],
                                             ej[:, i * TS:(i + 1) * TS],
                                             start=(j == 1), stop=False)
                ej = e_hold.pop(7)
                for i in range(2):
                    h = 2 * hp + i
                    nc.tensor.matmul(yps[i][:],
                                     v_sb[7][:, 65 * h:65 * h + 65],
                                     ej[:, i * TS:(i + 1) * TS],
                                     start=False, stop=True)
                pending = (hp, yps)
            softmax_epilogue(*pending)
            es_b.close()

            # ---- out-proj (token-local) + residual ----
            es_c = ExitStack()
            psO = es_c.enter_context(tc.tile_pool(name="psO", bufs=2, space="PSUM"))
            atpb_bc = bcast_row(lnrow, atpb[layer], 128, BF16, E, "atpb")
            for t in range(2):
                ps = psO.tile([128, E], F32, tag="o", name="o_ps")
                for n0, n1 in ((0, 512), (512, 768)):
                    for h in range(H):
                        nc.tensor.matmul(ps[:, n0:n1],
                                         yT2[h // 2][:, (h % 2) * TS + t * 128:
                                                     (h % 2) * TS + (t + 1) * 128],
                                         watp_sb[h][:, n0:n1],
                                         start=(h == 0), stop=(h == H - 1))
                nc.vector.tensor_add(out=x_sb[t][:], in0=x_sb[t][:], in1=ps[:])
                nc.vector.tensor_add(out=x_sb[t][:], in0=x_sb[t][:], in1=atpb_bc[:])
            es_c.close()

            unscope()
            scope(f"L{layer}_mlp")
            # ---- LN2 (pure normalize) + transpose ----
            h2T = [hTp.tile([128, TS], BF16, tag=f"hT{k}", name=f"hT{k}") for k in range(6)]
            h2_t = [hpool.tile([128, E], F32R, tag="h", name="h") for t in range(2)]
            layernorm2(x_sb, h2_t)
            for t in range(2):
                transpose_to(h2_t[t], h2T, t * 128)

            # ---- MLP fc: mT[m] = gelu(fc_w.T @ h2T + fc_b) ----
            es_d = ExitStack()
            psM = es_d.enter_context(tc.tile_pool(name="psM", bufs=2, space="PSUM"))
            psP = es_d.enter_context(tc.tile_pool(name="psP", bufs=1, space="PSUM"))
            mT = [mTp.tile([128, TS], BF16, tag=f"mT{m}", name=f"mT{m}") for m in range(24)]
            for m in range(24):
                ps = psM.tile([128, TS], F32, tag="m", name="m_ps")
                for k in range(6):
                    nc.tensor.matmul(ps[:], fck[k][:, m * 128:(m + 1) * 128],
                                     h2T[k][:], start=(k == 0), stop=(k == 5))
                nc.scalar.activation(out=mT[m][:], in_=ps[:],
                                     func=mybir.ActivationFunctionType.Gelu_apprx_tanh,
                                     bias=fcb_sb[:, m:m + 1])

            # ---- MLP pr + residual ----
            prb_bc = bcast_row(lnrow, prb[layer], 128, BF16, E, "prb")
            ps2 = [psP.tile([128, E], F32, tag=f"p{t}", name=f"p{t}") for t in range(2)]
            for kk in range(24):
                prw_sb = prwp.tile([128, E], BF16, tag="prw", name="prw")
                nc.sync.dma_start(out=prw_sb[:],
                                  in_=prw[layer, kk * 128:(kk + 1) * 128, :])
                for t in range(2):
                    for n0, n1 in ((0, 512), (512, 768)):
                        nc.tensor.matmul(ps2[t][:, n0:n1],
                                         mT[kk][:, t * 128:(t + 1) * 128],
                                         prw_sb[:, n0:n1],
                                         start=(kk == 0), stop=(kk == 23))
            for t in range(2):
                nc.vector.tensor_add(out=x_sb[t][:], in0=x_sb[t][:], in1=ps2[t][:])
                nc.vector.tensor_add(out=x_sb[t][:], in0=x_sb[t][:], in1=prb_bc[:])
            es_d.close()
            unscope()
            wcur = wnext

        # ---- final LN (affine folded into wte) + AllGather(all 8) ----
        scope("lnf_ag")
        xfT = [hTp.tile([128, TS], BF16, tag=f"hT{k}", name=f"hT{k}") for k in range(6)]
        xf_t = [hpool.tile([128, E], F32R, tag="h", name="h") for t in range(2)]
        layernorm2(x_sb, xf_t)
        for t in range(2):
            transpose_to(xf_t[t], xfT, t * 128)
        for k in range(6):
            nc.sync.dma_start(out=xf_in[k * 128:(k + 1) * 128, :], in_=xfT[k][:])
        es_l.close()

        es_h = es.enter_context(ExitStack())
        xfp = es_h.enter_context(tc.tile_pool(name="xfp", bufs=1))
        wtep = es_h.enter_context(tc.tile_pool(name="wtep", bufs=3))
        lop = es_h.enter_context(tc.tile_pool(name="lop", bufs=3))
        psL = es_h.enter_context(tc.tile_pool(name="psL", bufs=2, space="PSUM"))

        NG = 5                      # wte groups
        GW = VS // NG               # 1280 vocab cols per group
        NVG = GW // 128             # 10 vocab tiles per group

        def load_wte_group(g):
            tiles = [wtep.tile([128, GW], BF16, tag=f"wte{k}", name=f"wte{k}")
                     for k in range(6)]
            for k in range(6):
                nc.sync.dma_start(out=tiles[k][:],
                                  in_=wteT[k * 128:(k + 1) * 128, g * GW:(g + 1) * GW])
            return tiles

        wte_g0 = load_wte_group(0)      # prefetch during the AllGather

        nc.gpsimd.collective_compute(
            "AllGather", mybir.AluOpType.bypass,
            replica_groups=g_all,
            ins=[xf_in.opt()],
            outs=[xf_ag.opt()],
        )
        xf_ag_b = xf_ag[:]
        xf_sb = [xfp.tile([128, NC * TS], BF16, tag=f"xf{k}", name=f"xf{k}")
                 for k in range(6)]
        for k in range(6):
            nc.sync.dma_start(
                out=xf_sb[k][:].rearrange("p (r t) -> p r t", r=NC),
                in_=bass.AP(tensor=xf_ag_b.tensor,
                            offset=xf_ag_b.offset + k * 128 * TS,
                            ap=[[TS, 128], [E * TS, NC], [1, TS]]))
        unscope()

        scope("lmhead")
        for g in range(NG):
            wte_g = wte_g0 if g == 0 else load_wte_group(g)
            for v in range(NVG):
                ps = psL.tile([128, B * T], F32, tag="l", name="l_ps")
                for k in range(6):
                    stat_w = wte_g[k][:, v * 128:(v + 1) * 128]
                    for n in range(4):
                        nc.tensor.matmul(ps[:, n * 512:(n + 1) * 512],
                                         stat_w,
                                         xf_sb[k][:, n * 512:(n + 1) * 512],
                                         start=(k == 0), stop=(k == 5))
                lo = lop.tile([128, B * T], F32, tag="lo", name="lo")
                nc.vector.tensor_copy(lo[:, 0:1024], ps[:, 0:1024])
                nc.scalar.activation(out=lo[:, 1024:2048], in_=ps[:, 1024:2048],
                                     func=mybir.ActivationFunctionType.Copy)
                vg = g * NVG + v
                nc.sync.dma_start(out=logits[vg * 128:(vg + 1) * 128, :], in_=lo[:])
        unscope()

    nc.compile()
    return nc


def _prep_inputs(idx, wte, wpe, ln1_w, ln1_b, attn_w, attn_b, atp_w, atp_b,
                 ln2_w, ln2_b, fc_w, fc_b, pr_w, pr_b, lnf_w, lnf_b):
    idx = np.asarray(idx)
    f = lambda a: np.ascontiguousarray(np.asarray(a), dtype=np.float32)
    bf = lambda a: np.ascontiguousarray(np.asarray(a, dtype=np.float32).astype(BF))
    wte32, wpe32 = f(wte), f(wpe)
    x0 = wte32[idx.reshape(-1)] + np.tile(wpe32[:T], (B, 1))  # [2048, 768]

    ln1_w, ln1_b = f(ln1_w), f(ln1_b)
    ln2_w, ln2_b = f(ln2_w), f(ln2_b)
    lnf_w, lnf_b = f(lnf_w), f(lnf_b)

    # fold LN affine into the following matmuls (exact)
    aw = f(attn_w)
    attn_b = f(attn_b) + np.einsum("le,lef->lf", ln1_b, aw)
    attn_w = aw * ln1_w[:, :, None]
    fw = f(fc_w)
    fc_b2 = f(fc_b) + np.einsum("le,lef->lf", ln2_b, fw)
    fc_w2 = fw * ln2_w[:, :, None]
    wteT_scaled = (wte32 * lnf_w[None, :]).T  # [E, V]
    logit_bias = lnf_b @ wte32.T              # [V]

    wte_pad = np.zeros((E, VPAD), np.float32)
    wte_pad[:, :V] = wteT_scaled
    wteT_full = wte_pad.astype(BF)

    common = {
        "wq": bf(attn_w[:, :, 0:E]),
        "bq": np.ascontiguousarray(attn_b[:, 0:E].reshape(L, H, HD)),
        "wk": bf(attn_w[:, :, E:2 * E]),
        "bk": np.ascontiguousarray(attn_b[:, E:2 * E].reshape(L, 6, 128)),
        "wv": bf(attn_w[:, :, 2 * E:3 * E]),
        "bv": bf(attn_b[:, 2 * E:3 * E]),
        "watp": bf(np.asarray(atp_w).reshape(L, H, HD, E)),
        "atpb": bf(atp_b),
        "fcw": bf(fc_w2.reshape(L, 6, 128, 4 * E)),
        "fcb": np.ascontiguousarray(fc_b2.reshape(L, 24, 128)),
        "prw": bf(pr_w), "prb": bf(pr_b),
    }
    in_maps = []
    kidx = np.arange(128)
    qidx = np.arange(TS)
    for c in range(NC):
        r = c % 4
        m = np.zeros((8, 128, TS), np.float32)
        for j in range(8):
            m[j] = ((128 * j + kidx)[:, None] <= (TS * r + qidx)[None, :])
        m2 = np.concatenate([m, m], axis=2)
        in_maps.append({
            **common,
            "x0s": np.ascontiguousarray(x0[c * TS:(c + 1) * TS]),
            "mask": m2.astype(BF),
            "wteT": np.ascontiguousarray(wteT_full[:, c * VS:(c + 1) * VS]),
        })
    return in_maps, logit_bias


def kernel(trace=False, **inputs):
    if "nc" not in _CACHE:
        _CACHE["nc"] = _build_program()
    nc = _CACHE["nc"]
    in_maps, logit_bias = _prep_inputs(**inputs)
    res = run_bass_kernel_spmd(nc, in_maps, core_ids=list(range(NC)), trace=trace)
    _CACHE["last_result"] = res
    logits = np.concatenate([res.results[c]["logits"] for c in range(NC)], axis=0)
    out = np.ascontiguousarray(logits[:V].T)  # [B*T, V]
    if np.any(logit_bias):
        out += logit_bias[None, :]
    return out.reshape(B, T, V).astype(np.float32)


# revision 23
# speedup vs baseline: 1.0711x; 1.0711x over previous
"""GPT (4-layer, E=768, H=12, T=1024, B=2, V=50257) forward on 8 trn2 cores.

Sharding (v4):
  - Token-parallel residual: core c owns tokens [r*256,(r+1)*256) of batch
    c//4 (r = c%4).  LN / QKV / out-proj / MLP fully token-local.
  - LN affine (gamma/beta) folded into the following matmul weights on the
    host (exact), so device LN is a pure normalize.
  - Attention: per layer ONE AllGather per batch group of 4 carrying both
    K (dim-major [768,256]) and V (token-major [256,780], softmax ones
    column pre-packed) in a single packed bf16 buffer.  Each core computes
    all 12 heads for its own 256 queries against the full 1024 keys with a
    per-core causal mask (masking by data, not control flow).
  - Softmax normalize: denominators from the V ones-column; fast approx
    reciprocal + PE K=1 broadcast matmul + one DVE multiply per head pair.
  - lm_head vocab-sharded with wte STATIONARY (1 LDWEIGHTS per 4 N=512
    matmuls): each core computes a [6400, 2048] logit slice (V padded to
    51200), host transposes/concats.
  - All matmul operands bf16 (fp32 PSUM accumulation).
"""

import sys
from contextlib import ExitStack
import numpy as np
import ml_dtypes

sys.path.insert(0, "/opt/trn_rl_repo")

import concourse.bass as bass
import concourse.mybir as mybir
import concourse.tile as tile
from concourse import bacc
from concourse.bass_utils import run_bass_kernel_spmd
from concourse.masks import make_identity

L, H, E, T, V = 4, 12, 768, 1024, 50257
B = 2
NC = 8
TS = (B * T) // NC          # 256 tokens per core
VS = 6400                   # vocab slice per core (padded V = 51200)
VPAD = VS * NC
HD = 64
EPS = 1e-5
SCALE = float(1.0 / np.sqrt(np.float32(E)))
F32 = mybir.dt.float32
F32R = mybir.dt.float32r
BF16 = mybir.dt.bfloat16
BF = ml_dtypes.bfloat16

KB_EL = E * TS              # K elems in packed kv buffer
VB_EL = TS * H * 65         # V elems
KVN = KB_EL + VB_EL

_CACHE = {}


def _build_program():
    nc = bacc.Bacc("TRN2", target_bir_lowering=False, debug=False, num_devices=NC)

    # ---- I/O -------------------------------------------------------------
    x0s = nc.dram_tensor("x0s", [TS, E], F32, kind="ExternalInput")
    wq = nc.dram_tensor("wq", [L, E, E], BF16, kind="ExternalInput")
    bq = nc.dram_tensor("bq", [L, H, HD], F32, kind="ExternalInput")
    wk = nc.dram_tensor("wk", [L, E, E], BF16, kind="ExternalInput")
    bk = nc.dram_tensor("bk", [L, 6, 128], F32, kind="ExternalInput")
    wv = nc.dram_tensor("wv", [L, E, E], BF16, kind="ExternalInput")
    bv = nc.dram_tensor("bv", [L, E], BF16, kind="ExternalInput")
    watp = nc.dram_tensor("watp", [L, H, HD, E], BF16, kind="ExternalInput")
    atpb = nc.dram_tensor("atpb", [L, E], BF16, kind="ExternalInput")
    fcw = nc.dram_tensor("fcw", [L, 6, 128, 4 * E], BF16, kind="ExternalInput")
    fcb = nc.dram_tensor("fcb", [L, 24, 128], F32, kind="ExternalInput")
    prw = nc.dram_tensor("prw", [L, 4 * E, E], BF16, kind="ExternalInput")
    prb = nc.dram_tensor("prb", [L, E], BF16, kind="ExternalInput")
    mask = nc.dram_tensor("mask", [8, 128, 2 * TS], BF16, kind="ExternalInput")
    wteT = nc.dram_tensor("wteT", [E, VS], BF16, kind="ExternalInput")
    logits = nc.dram_tensor("logits", [VS, B * T], F32, kind="ExternalOutput")

    g_all = [list(range(NC))]
    g_batch = [[0, 1, 2, 3], [4, 5, 6, 7]]

    def bcast_row(pool, src_ap, n, dtype, w, name="bc"):
        """Replicate a [w] DRAM row across n partitions via broadcast DMA."""
        t = pool.tile([n, w], dtype, tag=name, name=name)
        in_ap = bass.AP(
            tensor=src_ap.tensor,
            offset=src_ap.offset,
            ap=[[0, n]] + [list(p) for p in src_ap.ap],
        )
        nc.sync.dma_start(out=t[:], in_=in_ap)
        return t

    with tile.TileContext(nc) as tc, ExitStack() as es:
        const = es.enter_context(tc.tile_pool(name="const", bufs=1))
        xp = es.enter_context(tc.tile_pool(name="xp", bufs=1))
        lnrow = es.enter_context(tc.tile_pool(name="lnrow", bufs=2))
        stat = es.enter_context(tc.tile_pool(name="stat", bufs=4))
        hpool = es.enter_context(tc.tile_pool(name="hpool", bufs=2))
        hTp = es.enter_context(tc.tile_pool(name="hTp", bufs=1))
        dram = es.enter_context(tc.tile_pool(name="dram", bufs=1, space="DRAM"))

        # warm up the collective rings while the first layer computes
        warm_in = dram.tile([256], BF16)
        warm_out1 = dram.tile([1024], BF16)
        warm_out2 = dram.tile([2048], BF16)
        nc.gpsimd.collective_compute(
            "AllGather", mybir.AluOpType.bypass, replica_groups=g_batch,
            ins=[warm_in[:]], outs=[warm_out1[:]])
        nc.gpsimd.collective_compute(
            "AllGather", mybir.AluOpType.bypass, replica_groups=g_all,
            ins=[warm_in[:]], outs=[warm_out2[:]])

        # persistent residual stream [256, 768] as two [128, 768] f32 tiles
        x_sb = [xp.tile([128, E], F32, tag=f"x{t}", name=f"x{t}") for t in range(2)]
        for t in range(2):
            nc.sync.dma_start(out=x_sb[t][:], in_=x0s[t * 128:(t + 1) * 128, :])

        ident_f = const.tile([128, 128], F32, name="ident_f")
        make_identity(nc, ident_f)
        ident = const.tile([128, 128], F32R, name="ident")
        nc.vector.tensor_copy(ident[:], ident_f[:])
        eps_sb = const.tile([128, 1], F32)
        nc.vector.memset(eps_sb, EPS)
        ones780 = const.tile([128, H * 65], BF16, name="ones780")
        nc.vector.memset(ones780, 1.0)
        mask_sb = [const.tile([128, 2 * TS], BF16, name=f"mask{j}") for j in range(8)]

        # DRAM bounce buffers for collectives (K and V packed in one buffer)
        kv_in = dram.tile([KVN], BF16)
        kv_ag = dram.tile([4 * KVN], BF16)
        xf_in = dram.tile([E, TS], BF16)
        xf_ag = dram.tile([NC * E, TS], BF16, addr_space="Shared")
        kv_in_b = kv_in[:]
        kv_ag_b = kv_ag[:]

        def layernorm2(x_tiles, out_tiles):
            """Pure normalize (no affine) of two [128,768] f32 tiles."""
            stats = stat.tile([128, 2, 3, 6], F32, tag="bn_stats", name="bn_stats_t")
            for t in range(2):
                xr = x_tiles[t][:].rearrange("p (s d) -> p s d", s=3)
                for s in range(3):
                    nc.vector.bn_stats(out=stats[:, t, s, :], in_=xr[:, s, :])
            mv = stat.tile([128, 2, 2], F32, tag="bn_aggr", name="bn_aggr_t")
            for t in range(2):
                nc.vector.bn_aggr(out=mv[:, t, :], in_=stats[:, t])
            std = stat.tile([128, 2], F32, tag="std", name="std_t")
            for t in range(2):
                nc.scalar.activation(out=std[:, t:t + 1], in_=mv[:, t, 1:2],
                                     func=mybir.ActivationFunctionType.Sqrt,
                                     bias=eps_sb[:], scale=1.0)
            rstd = stat.tile([128, 2], F32, tag="rstd", name="rstd_t")
            nc.vector.reciprocal_approx_fast(out=rstd[:], in_=std[:])
            for t in range(2):
                nc.vector.tensor_scalar(out=out_tiles[t][:], in0=x_tiles[t][:],
                                        scalar1=mv[:, t, 0:1],
                                        scalar2=rstd[:, t:t + 1],
                                        op0=mybir.AluOpType.subtract,
                                        op1=mybir.AluOpType.mult)

        def scope(name):
            sid, _ = nc.enter_named_scope(name, False)
            _SCOPES.append((name, sid))

        def unscope():
            name, sid = _SCOPES.pop()
            nc.leave_named_scope(name, sid, False)

        _SCOPES = []

        es_l = es.enter_context(ExitStack())
        wqp = es_l.enter_context(tc.tile_pool(name="wqp", bufs=1))
        wkp = es_l.enter_context(tc.tile_pool(name="wkp", bufs=1))
        wvp = es_l.enter_context(tc.tile_pool(name="wvp", bufs=1))
        watpp = es_l.enter_context(tc.tile_pool(name="watpp", bufs=1))
        bias_p = es_l.enter_context(tc.tile_pool(name="bias_p", bufs=2))
        qTp = es_l.enter_context(tc.tile_pool(name="qTp", bufs=1))
        kvsp = es_l.enter_context(tc.tile_pool(name="kvsp", bufs=2))
        kTp = es_l.enter_context(tc.tile_pool(name="kTp", bufs=1))
        vp = es_l.enter_context(tc.tile_pool(name="vp", bufs=1))
        ep = es_l.enter_context(tc.tile_pool(name="ep", bufs=4))
        yp = es_l.enter_context(tc.tile_pool(name="yp", bufs=1))
        sm = es_l.enter_context(tc.tile_pool(name="sm", bufs=2))
        fcwp = es_l.enter_context(tc.tile_pool(name="fcwp", bufs=1))
        mTp = es_l.enter_context(tc.tile_pool(name="mTp", bufs=1))
        prwp = es_l.enter_context(tc.tile_pool(name="prwp", bufs=5))
        psT = es_l.enter_context(tc.tile_pool(name="psT", bufs=2, space="PSUM"))

        def transpose_to(hsrc, dst_tiles, dst_col):
            """hsrc [128,768] f32r -> bf16 dst_tiles[k][:, dst_col:dst_col+128]."""
            for k in range(6):
                pt = psT.tile([128, 128], F32R, tag="tr", name="tr")
                nc.tensor.transpose(pt[:], hsrc[:, k * 128:(k + 1) * 128], ident[:])
                dst = dst_tiles[k][:, dst_col:dst_col + 128]
                if k % 2 == 0:
                    nc.vector.tensor_copy(dst, pt[:].bitcast(F32))
                else:
                    nc.scalar.activation(out=dst, in_=pt[:].bitcast(F32),
                                         func=mybir.ActivationFunctionType.Copy)

        def load_kv_weights(layer):
            w = {}
            w["wk"] = [wkp.tile([128, E], BF16, tag=f"wk{k}", name=f"wk{k}")
                       for k in range(6)]
            for k in range(6):
                nc.sync.dma_start(out=w["wk"][k][:],
                                  in_=wk[layer, k * 128:(k + 1) * 128, :])
            w["bk"] = bias_p.tile([128, 6], F32, tag="bk", name="bk")
            nc.sync.dma_start(out=w["bk"][:], in_=bk[layer].transpose([1, 0]))
            w["wv"] = [wvp.tile([128, E], BF16, tag=f"wv{k}", name=f"wv{k}")
                       for k in range(6)]
            for k in range(6):
                nc.sync.dma_start(out=w["wv"][k][:],
                                  in_=wv[layer, k * 128:(k + 1) * 128, :])
            return w

        wcur = load_kv_weights(0)
        for j in range(8):
            nc.sync.dma_start(out=mask_sb[j][:], in_=mask[j])

        for layer in range(L):
            scope(f"L{layer}_qkv")
            wk_sb, wv_sb = wcur["wk"], wcur["wv"]
            bk_sb = wcur["bk"]
            # ---- LN1 (pure normalize) + transpose -> hT [768, 256] bf16 ----
            hT = [hTp.tile([128, TS], BF16, tag=f"hT{k}", name=f"hT{k}") for k in range(6)]
            h_t = [hpool.tile([128, E], F32R, tag="h", name="h") for t in range(2)]
            layernorm2(x_sb, h_t)
            for t in range(2):
                transpose_to(h_t[t], hT, t * 128)

            bv_bc = bcast_row(lnrow, bv[layer], 128, BF16, E, "bv")

            es_a = ExitStack()
            psK = es_a.enter_context(tc.tile_pool(name="psK", bufs=2, space="PSUM"))
            psV = es_a.enter_context(tc.tile_pool(name="psV", bufs=1, space="PSUM"))
            for m in range(6):
                ps = psK.tile([128, TS], F32, tag="kps", name="k_ps")
                for k in range(6):
                    nc.tensor.matmul(ps[:], wk_sb[k][:, m * 128:(m + 1) * 128],
                                     hT[k][:], start=(k == 0), stop=(k == 5))
                kt = kvsp.tile([128, TS], BF16, tag="kloc", name="kloc")
                nc.vector.tensor_scalar_add(out=kt[:], in0=ps[:],
                                            scalar1=bk_sb[:, m:m + 1])
                nc.sync.dma_start(
                    out=bass.AP(tensor=kv_in_b.tensor,
                                offset=kv_in_b.offset + m * 128 * TS,
                                ap=[[TS, 128], [1, TS]]),
                    in_=kt[:])

            # ---- V (token-major, ones column pre-packed) ----
            for t in range(2):
                ps = psV.tile([128, E], F32, tag="vps", name="v_ps")
                for n0, n1 in ((0, 512), (512, 768)):
                    for k in range(6):
                        nc.tensor.matmul(ps[:, n0:n1],
                                         hT[k][:, t * 128:(t + 1) * 128],
                                         wv_sb[k][:, n0:n1],
                                         start=(k == 0), stop=(k == 5))
                vt = kvsp.tile([128, H * 65], BF16, tag="vloc", name="vloc")
                nc.vector.tensor_copy(vt[:], ones780[:])
                for h in range(H):
                    nc.vector.tensor_add(out=vt[:, 65 * h:65 * h + 64],
                                         in0=ps[:, 64 * h:64 * h + 64],
                                         in1=bv_bc[:, 64 * h:64 * h + 64])
                nc.sync.dma_start(
                    out=bass.AP(tensor=kv_in_b.tensor,
                                offset=kv_in_b.offset + KB_EL + t * 128 * (H * 65),
                                ap=[[H * 65, 128], [1, H * 65]]),
                    in_=vt[:])
            nc.gpsimd.collective_compute(
                "AllGather", mybir.AluOpType.bypass,
                replica_groups=g_batch,
                ins=[kv_in[:]],
                outs=[kv_ag[:]],
            )

            # ---- prefetch proj/MLP weights into the AllGather window ----
            watp_sb = [watpp.tile([64, E], BF16, tag=f"watp{h}", name=f"watp{h}")
                       for h in range(H)]
            for h in range(H):
                nc.sync.dma_start(out=watp_sb[h][:], in_=watp[layer, h])
            fck = [fcwp.tile([128, 4 * E], BF16, tag=f"fck{k}", name=f"fck{k}")
                   for k in range(6)]
            for k in range(6):
                nc.sync.dma_start(out=fck[k][:], in_=fcw[layer, k])
            fcb_sb = bias_p.tile([128, 24], F32, tag="fcb", name="fcb")
            nc.sync.dma_start(out=fcb_sb[:], in_=fcb[layer].transpose([1, 0]))

            # ---- Q per head (overlaps the AllGather) ----
            wq_sb = [wqp.tile([128, E], BF16, tag=f"wq{k}", name=f"wq{k}")
                     for k in range(6)]
            for k in range(6):
                nc.sync.dma_start(out=wq_sb[k][:],
                                  in_=wq[layer, k * 128:(k + 1) * 128, :])
            bq_sb = bias_p.tile([64, H], F32, tag="bq", name="bq")
            nc.sync.dma_start(out=bq_sb[:], in_=bq[layer].transpose([1, 0]))
            psQ = es_a.enter_context(tc.tile_pool(name="psQ", bufs=2, space="PSUM"))
            qT = [qTp.tile([64, TS], BF16, tag=f"qT{h}", name=f"qT{h}") for h in range(H)]
            for h in range(H):
                ps = psQ.tile([64, TS], F32, tag="q", name="q_ps")
                for k in range(6):
                    nc.tensor.matmul(ps[:], wq_sb[k][:, h * 64:(h + 1) * 64],
                                     hT[k][:], start=(k == 0), stop=(k == 5))
                nc.vector.tensor_scalar_add(out=qT[h][:], in0=ps[:],
                                            scalar1=bq_sb[:, h:h + 1])
            es_a.close()

            unscope()
            scope(f"L{layer}_attn")
            # ---- load gathered K (per-head) and V from the packed buffer ----
            kTh = [kTp.tile([64, T], BF16, tag=f"kTh{h}", name=f"kTh{h}")
                   for h in range(H)]
            v_sb = [vp.tile([128, H * 65], BF16, tag=f"v{j}", name=f"v{j}")
                    for j in range(8)]

            def load_kth(h):
                nc.sync.dma_start(
                    out=kTh[h][:].rearrange("p (r c) -> p r c", r=4),
                    in_=bass.AP(tensor=kv_ag_b.tensor,
                                offset=kv_ag_b.offset + h * 64 * TS,
                                ap=[[TS, 64], [KVN, 4], [1, TS]]))

            def load_v(j):
                nc.sync.dma_start(
                    out=v_sb[j][:],
                    in_=bass.AP(tensor=kv_ag_b.tensor,
                                offset=(kv_ag_b.offset + (j // 2) * KVN + KB_EL
                                        + (j % 2) * 128 * (H * 65)),
                                ap=[[H * 65, 128], [1, H * 65]]))

            load_kth(0)
            load_kth(1)
            for j in range(8):
                load_v(j)
            for h in range(2, H):
                load_kth(h)

            # prefetch next layer's K/V weights (behind the kv loads in priority)
            wnext = load_kv_weights(layer + 1) if layer + 1 < L else None

            # ---- attention: 12 heads, 256 queries vs 1024 masked keys ----
            es_b = ExitStack()
            psS = es_b.enter_context(tc.tile_pool(name="psS", bufs=2, space="PSUM"))
            psY = es_b.enter_context(tc.tile_pool(name="psY", bufs=2, space="PSUM"))
            yT2 = [yp.tile([64, 2 * TS], BF16, tag=f"yT{hp}", name=f"yT{hp}")
                   for hp in range(6)]

            def softmax_epilogue(hp, yps):
                # den rows -> approx recip -> PE broadcast -> mul
                den = sm.tile([1, 2 * TS], F32, tag="den", name="den")
                for i in range(2):
                    nc.vector.tensor_copy(den[:, i * TS:(i + 1) * TS],
                                          yps[i][64:65, :])
                rec = sm.tile([1, 2 * TS], F32, tag="rec", name="rec")
                nc.vector.reciprocal_approx_fast(out=rec[:], in_=den[:])
                rec_bf = sm.tile([1, 2 * TS], BF16, tag="rec_bf", name="rec_bf")
                nc.vector.tensor_copy(rec_bf[:], rec[:])
                psb = psT.tile([64, 2 * TS], F32, tag="tr", name="bc_ps")
                nc.tensor.matmul(psb[:], ones780[0:1, 0:64], rec_bf[:],
                                 start=True, stop=True)
                rbc = sm.tile([64, 2 * TS], BF16, tag="rbc", name="rbc")
                nc.vector.tensor_copy(rbc[:], psb[:])
                for i in range(2):
                    nc.vector.tensor_mul(out=yT2[hp][:, i * TS:(i + 1) * TS],
                                         in0=yps[i][0:64, :],
                                         in1=rbc[:, i * TS:(i + 1) * TS])

            # software-pipelined: scores for j+1 issue before AV for j, and
            # the previous head-pair's epilogue issues after this pair's
            # first scores, so the PE never waits on the exp/mask chain.
            pending = None
            for hp in range(6):
                yps = [psY.tile([65, TS], F32, tag=f"y{i}", name=f"y{i}")
                       for i in range(2)]
                e_hold = {}

                def av(j, last):
                    ej = e_hold.pop(j)
                    for i in range(2):
                        h = 2 * hp + i
                        nc.tensor.matmul(yps[i][:],
                                         v_sb[j][:, 65 * h:65 * h + 65],
                                         ej[:, i * TS:(i + 1) * TS],
                                         start=(j == 0), stop=last)

                for j in range(8):
                    pss = psS.tile([128, 2 * TS], F32, tag="s", name="s_ps")
                    e_sb = ep.tile([128, 2 * TS], BF16, tag="e", name="e")
                    for i in range(2):
                        nc.tensor.matmul(pss[:, i * TS:(i + 1) * TS],
                                         kTh[2 * hp + i][:, j * 128:(j + 1) * 128],
                                         qT[2 * hp + i][:], start=True, stop=True)
                    nc.scalar.activation(out=e_sb[:], in_=pss[:],
                                         func=mybir.ActivationFunctionType.Exp,
                                         scale=SCALE)
                    nc.vector.tensor_mul(out=e_sb[:], in0=e_sb[:], in1=mask_sb[j][:])
                    e_hold[j] = e_sb
                    if j == 2 and pending is not None:
                        softmax_epilogue(*pending)
                        pending = None
                    if j >= 2:
                        av(j - 2, False)
                av(6, False)
                av(7, True)
                pending = (hp, yps)
            softmax_epilogue(*pending)
            es_b.close()

            # ---- out-proj (token-local) + residual ----
            es_c = ExitStack()
            psO = es_c.enter_context(tc.tile_pool(name="psO", bufs=2, space="PSUM"))
            atpb_bc = bcast_row(lnrow, atpb[layer], 128, BF16, E, "atpb")
            for t in range(2):
                ps = psO.tile([128, E], F32, tag="o", name="o_ps")
                for n0, n1 in ((0, 512), (512, 768)):
                    for h in range(H):
                        nc.tensor.matmul(ps[:, n0:n1],
                                         yT2[h // 2][:, (h % 2) * TS + t * 128:
                                                     (h % 2) * TS + (t + 1) * 128],
                                         watp_sb[h][:, n0:n1],
                                         start=(h == 0), stop=(h == H - 1))
                nc.vector.tensor_add(out=x_sb[t][:], in0=x_sb[t][:], in1=ps[:])
                nc.vector.tensor_add(out=x_sb[t][:], in0=x_sb[t][:], in1=atpb_bc[:])
            es_c.close()

            unscope()
            scope(f"L{layer}_mlp")
            # ---- LN2 (pure normalize) + transpose ----
            h2T = [hTp.tile([128, TS], BF16, tag=f"hT{k}", name=f"hT{k}") for k in range(6)]
            h2_t = [hpool.tile([128, E], F32R, tag="h", name="h") for t in range(2)]
            layernorm2(x_sb, h2_t)
            for t in range(2):
                transpose_to(h2_t[t], h2T, t * 128)

            # ---- MLP fc: mT[m] = gelu(fc_w.T @ h2T + fc_b) ----
            es_d = ExitStack()
            psM = es_d.enter_context(tc.tile_pool(name="psM", bufs=2, space="PSUM"))
            psP = es_d.enter_context(tc.tile_pool(name="psP", bufs=1, space="PSUM"))
            mT = [mTp.tile([128, TS], BF16, tag=f"mT{m}", name=f"mT{m}") for m in range(24)]
            for m in range(24):
                ps = psM.tile([128, TS], F32, tag="m", name="m_ps")
                for k in range(6):
                    nc.tensor.matmul(ps[:], fck[k][:, m * 128:(m + 1) * 128],
                                     h2T[k][:], start=(k == 0), stop=(k == 5))
                nc.scalar.activation(out=mT[m][:], in_=ps[:],
                                     func=mybir.ActivationFunctionType.Gelu_apprx_tanh,
                                     bias=fcb_sb[:, m:m + 1])

            # ---- MLP pr + residual ----
            prb_bc = bcast_row(lnrow, prb[layer], 128, BF16, E, "prb")
            ps2 = [psP.tile([128, E], F32, tag=f"p{t}", name=f"p{t}") for t in range(2)]
            for kk in range(24):
                prw_sb = prwp.tile([128, E], BF16, tag="prw", name="prw")
                nc.sync.dma_start(out=prw_sb[:],
                                  in_=prw[layer, kk * 128:(kk + 1) * 128, :])
                for t in range(2):
                    for n0, n1 in ((0, 512), (512, 768)):
                        nc.tensor.matmul(ps2[t][:, n0:n1],
                                         mT[kk][:, t * 128:(t + 1) * 128],
                                         prw_sb[:, n0:n1],
                                         start=(kk == 0), stop=(kk == 23))
            for t in range(2):
                nc.vector.tensor_add(out=x_sb[t][:], in0=x_sb[t][:], in1=ps2[t][:])
                nc.vector.tensor_add(out=x_sb[t][:], in0=x_sb[t][:], in1=prb_bc[:])
            es_d.close()
            unscope()
            wcur = wnext

        # ---- final LN (affine folded into wte) + AllGather(all 8) ----
        scope("lnf_ag")
        xfT = [hTp.tile([128, TS], BF16, tag=f"hT{k}", name=f"hT{k}") for k in range(6)]
        xf_t = [hpool.tile([128, E], F32R, tag="h", name="h") for t in range(2)]
        layernorm2(x_sb, xf_t)
        for t in range(2):
            transpose_to(xf_t[t], xfT, t * 128)
        for k in range(6):
            nc.sync.dma_start(out=xf_in[k * 128:(k + 1) * 128, :], in_=xfT[k][:])
        es_l.close()

        es_h = es.enter_context(ExitStack())
        xfp = es_h.enter_context(tc.tile_pool(name="xfp", bufs=1))
        wtep = es_h.enter_context(tc.tile_pool(name="wtep", bufs=3))
        lop = es_h.enter_context(tc.tile_pool(name="lop", bufs=3))
        psL = es_h.enter_context(tc.tile_pool(name="psL", bufs=2, space="PSUM"))

        NG = 5                      # wte groups
        GW = VS // NG               # 1280 vocab cols per group
        NVG = GW // 128             # 10 vocab tiles per group

        def load_wte_group(g):
            tiles = [wtep.tile([128, GW], BF16, tag=f"wte{k}", name=f"wte{k}")
                     for k in range(6)]
            for k in range(6):
                nc.sync.dma_start(out=tiles[k][:],
                                  in_=wteT[k * 128:(k + 1) * 128, g * GW:(g + 1) * GW])
            return tiles

        wte_g0 = load_wte_group(0)      # prefetch during the AllGather

        nc.gpsimd.collective_compute(
            "AllGather", mybir.AluOpType.bypass,
            replica_groups=g_all,
            ins=[xf_in.opt()],
            outs=[xf_ag.opt()],
        )
        xf_ag_b = xf_ag[:]
        xf_sb = [xfp.tile([128, NC * TS], BF16, tag=f"xf{k}", name=f"xf{k}")
                 for k in range(6)]
        for k in range(6):
            nc.sync.dma_start(
                out=xf_sb[k][:].rearrange("p (r t) -> p r t", r=NC),
                in_=bass.AP(tensor=xf_ag_b.tensor,
                            offset=xf_ag_b.offset + k * 128 * TS,
                            ap=[[TS, 128], [E * TS, NC], [1, TS]]))
        unscope()

        scope("lmhead")
        for g in range(NG):
            wte_g = wte_g0 if g == 0 else load_wte_group(g)
            for v in range(NVG):
                ps = psL.tile([128, B * T], F32, tag="l", name="l_ps")
                for k in range(6):
                    stat_w = wte_g[k][:, v * 128:(v + 1) * 128]
                    for n in range(4):
                        nc.tensor.matmul(ps[:, n * 512:(n + 1) * 512],
                                         stat_w,
                                         xf_sb[k][:, n * 512:(n + 1) * 512],
                                         start=(k == 0), stop=(k == 5))
                lo = lop.tile([128, B * T], F32, tag="lo", name="lo")
                nc.vector.tensor_copy(lo[:, 0:1024], ps[:, 0:1024])
                nc.scalar.activation(out=lo[:, 1024:2048], in_=ps[:, 1024:2048],
                                     func=mybir.ActivationFunctionType.Copy)
                vg = g * NVG + v
                nc.sync.dma_start(out=logits[vg * 128:(vg + 1) * 128, :], in_=lo[:])
        unscope()

    nc.compile()
    return nc


def _prep_inputs(idx, wte, wpe, ln1_w, ln1_b, attn_w, attn_b, atp_w, atp_b,
                 ln2_w, ln2_b, fc_w, fc_b, pr_w, pr_b, lnf_w, lnf_b):
    idx = np.asarray(idx)
    f = lambda a: np.ascontiguousarray(np.asarray(a), dtype=np.float32)
    bf = lambda a: np.ascontiguousarray(np.asarray(a, dtype=np.float32).astype(BF))
    wte32, wpe32 = f(wte), f(wpe)
    x0 = wte32[idx.reshape(-1)] + np.tile(wpe32[:T], (B, 1))  # [2048, 768]

    ln1_w, ln1_b = f(ln1_w), f(ln1_b)
    ln2_w, ln2_b = f(ln2_w), f(ln2_b)
    lnf_w, lnf_b = f(lnf_w), f(lnf_b)

    # fold LN affine into the following matmuls (exact)
    aw = f(attn_w)
    attn_b = f(attn_b) + np.einsum("le,lef->lf", ln1_b, aw)
    attn_w = aw * ln1_w[:, :, None]
    fw = f(fc_w)
    fc_b2 = f(fc_b) + np.einsum("le,lef->lf", ln2_b, fw)
    fc_w2 = fw * ln2_w[:, :, None]
    wteT_scaled = (wte32 * lnf_w[None, :]).T  # [E, V]
    logit_bias = lnf_b @ wte32.T              # [V]

    wte_pad = np.zeros((E, VPAD), np.float32)
    wte_pad[:, :V] = wteT_scaled
    wteT_full = wte_pad.astype(BF)

    common = {
        "wq": bf(attn_w[:, :, 0:E]),
        "bq": np.ascontiguousarray(attn_b[:, 0:E].reshape(L, H, HD)),
        "wk": bf(attn_w[:, :, E:2 * E]),
        "bk": np.ascontiguousarray(attn_b[:, E:2 * E].reshape(L, 6, 128)),
        "wv": bf(attn_w[:, :, 2 * E:3 * E]),
        "bv": bf(attn_b[:, 2 * E:3 * E]),
        "watp": bf(np.asarray(atp_w).reshape(L, H, HD, E)),
        "atpb": bf(atp_b),
        "fcw": bf(fc_w2.reshape(L, 6, 128, 4 * E)),
        "fcb": np.ascontiguousarray(fc_b2.reshape(L, 24, 128)),
        "prw": bf(pr_w), "prb": bf(pr_b),
    }
    in_maps = []
    kidx = np.arange(128)
    qidx = np.arange(TS)
    for c in range(NC):
        r = c % 4
        m = np.zeros((8, 128, TS), np.float32)
        for j in range(8):
            m[j] = ((128 * j + kidx)[:, None] <= (TS * r + qidx)[None, :])
        m2 = np.concatenate([m, m], axis=2)
        in_maps.append({
            **common,
            "x0s": np.ascontiguousarray(x0[c * TS:(c + 1) * TS]),
            "mask": m2.astype(BF),
            "wteT": np.ascontiguousarray(wteT_full[:, c * VS:(c + 1) * VS]),
        })
    return in_maps, logit_bias


def kernel(trace=False, **inputs):
    if "nc" not in _CACHE:
        _CACHE["nc"] = _build_program()
    nc = _CACHE["nc"]
    in_maps, logit_bias = _prep_inputs(**inputs)
    res = run_bass_kernel_spmd(nc, in_maps, core_ids=list(range(NC)), trace=trace)
    _CACHE["last_result"] = res
    logits = np.concatenate([res.results[c]["logits"] for c in range(NC)], axis=0)
    out = np.ascontiguousarray(logits[:V].T)  # [B*T, V]
    if np.any(logit_bias):
        out += logit_bias[None, :]
    return out.reshape(B, T, V).astype(np.float32)


# revision 25
# speedup vs baseline: 1.0759x; 1.0045x over previous
"""GPT (4-layer, E=768, H=12, T=1024, B=2, V=50257) forward on 8 trn2 cores.

Sharding (v4):
  - Token-parallel residual: core c owns tokens [r*256,(r+1)*256) of batch
    c//4 (r = c%4).  LN / QKV / out-proj / MLP fully token-local.
  - LN affine (gamma/beta) folded into the following matmul weights on the
    host (exact), so device LN is a pure normalize.
  - Attention: per layer ONE AllGather per batch group of 4 carrying both
    K (dim-major [768,256]) and V (token-major [256,780], softmax ones
    column pre-packed) in a single packed bf16 buffer.  Each core computes
    all 12 heads for its own 256 queries against the full 1024 keys with a
    per-core causal mask (masking by data, not control flow).
  - Softmax normalize: denominators from the V ones-column; fast approx
    reciprocal + PE K=1 broadcast matmul + one DVE multiply per head pair.
  - lm_head vocab-sharded with wte STATIONARY (1 LDWEIGHTS per 4 N=512
    matmuls): each core computes a [6400, 2048] logit slice (V padded to
    51200), host transposes/concats.
  - All matmul operands bf16 (fp32 PSUM accumulation).
"""

import sys
from contextlib import ExitStack
import numpy as np
import ml_dtypes

sys.path.insert(0, "/opt/trn_rl_repo")

import concourse.bass as bass
import concourse.mybir as mybir
import concourse.tile as tile
from concourse import bacc
from concourse.bass_utils import run_bass_kernel_spmd
from concourse.masks import make_identity

L, H, E, T, V = 4, 12, 768, 1024, 50257
B = 2
NC = 8
TS = (B * T) // NC          # 256 tokens per core
VS = 6400                   # vocab slice per core (padded V = 51200)
VPAD = VS * NC
HD = 64
EPS = 1e-5
SCALE = float(1.0 / np.sqrt(np.float32(E)))
F32 = mybir.dt.float32
F32R = mybir.dt.float32r
BF16 = mybir.dt.bfloat16
BF = ml_dtypes.bfloat16

KB_EL = E * TS              # K elems in packed kv buffer
VB_EL = TS * H * 65         # V elems
KVN = KB_EL + VB_EL

_CACHE = {}


def _build_program():
    nc = bacc.Bacc("TRN2", target_bir_lowering=False, debug=False, num_devices=NC)

    # ---- I/O -------------------------------------------------------------
    x0s = nc.dram_tensor("x0s", [TS, E], F32, kind="ExternalInput")
    wq = nc.dram_tensor("wq", [L, E, E], BF16, kind="ExternalInput")
    bq = nc.dram_tensor("bq", [L, H, HD], F32, kind="ExternalInput")
    wk = nc.dram_tensor("wk", [L, E, E], BF16, kind="ExternalInput")
    bk = nc.dram_tensor("bk", [L, 6, 128], F32, kind="ExternalInput")
    wv = nc.dram_tensor("wv", [L, E, E], BF16, kind="ExternalInput")
    bv = nc.dram_tensor("bv", [L, E], BF16, kind="ExternalInput")
    watp = nc.dram_tensor("watp", [L, H, HD, E], BF16, kind="ExternalInput")
    atpb = nc.dram_tensor("atpb", [L, E], BF16, kind="ExternalInput")
    fcw = nc.dram_tensor("fcw", [L, 6, 128, 4 * E], BF16, kind="ExternalInput")
    fcb = nc.dram_tensor("fcb", [L, 24, 128], F32, kind="ExternalInput")
    prw = nc.dram_tensor("prw", [L, 4 * E, E], BF16, kind="ExternalInput")
    prb = nc.dram_tensor("prb", [L, E], BF16, kind="ExternalInput")
    mask = nc.dram_tensor("mask", [8, 128, 2 * TS], BF16, kind="ExternalInput")
    wteT = nc.dram_tensor("wteT", [E, VS], BF16, kind="ExternalInput")
    logits = nc.dram_tensor("logits", [VS, B * T], F32, kind="ExternalOutput")

    g_all = [list(range(NC))]
    g_batch = [[0, 1, 2, 3], [4, 5, 6, 7]]

    def bcast_row(pool, src_ap, n, dtype, w, name="bc"):
        """Replicate a [w] DRAM row across n partitions via broadcast DMA."""
        t = pool.tile([n, w], dtype, tag=name, name=name)
        in_ap = bass.AP(
            tensor=src_ap.tensor,
            offset=src_ap.offset,
            ap=[[0, n]] + [list(p) for p in src_ap.ap],
        )
        nc.sync.dma_start(out=t[:], in_=in_ap)
        return t

    with tile.TileContext(nc) as tc, ExitStack() as es:
        const = es.enter_context(tc.tile_pool(name="const", bufs=1))
        xp = es.enter_context(tc.tile_pool(name="xp", bufs=1))
        lnrow = es.enter_context(tc.tile_pool(name="lnrow", bufs=2))
        stat = es.enter_context(tc.tile_pool(name="stat", bufs=4))
        hpool = es.enter_context(tc.tile_pool(name="hpool", bufs=2))
        hTp = es.enter_context(tc.tile_pool(name="hTp", bufs=1))
        dram = es.enter_context(tc.tile_pool(name="dram", bufs=1, space="DRAM"))

        # warm up the collective rings while the first layer computes
        warm_in = dram.tile([256], BF16)
        warm_out1 = dram.tile([1024], BF16)
        warm_out2 = dram.tile([2048], BF16)
        nc.gpsimd.collective_compute(
            "AllGather", mybir.AluOpType.bypass, replica_groups=g_batch,
            ins=[warm_in[:]], outs=[warm_out1[:]])

        # persistent residual stream [256, 768] as two [128, 768] f32 tiles
        x_sb = [xp.tile([128, E], F32, tag=f"x{t}", name=f"x{t}") for t in range(2)]
        for t in range(2):
            nc.sync.dma_start(out=x_sb[t][:], in_=x0s[t * 128:(t + 1) * 128, :])

        ident_f = const.tile([128, 128], F32, name="ident_f")
        make_identity(nc, ident_f)
        ident = const.tile([128, 128], F32R, name="ident")
        nc.vector.tensor_copy(ident[:], ident_f[:])
        eps_sb = const.tile([128, 1], F32)
        nc.vector.memset(eps_sb, EPS)
        ones780 = const.tile([128, H * 65], BF16, name="ones780")
        nc.vector.memset(ones780, 1.0)
        mask_sb = [const.tile([128, 2 * TS], BF16, name=f"mask{j}") for j in range(8)]

        # DRAM bounce buffers for collectives (K and V packed in one buffer)
        kv_in = dram.tile([KVN], BF16)
        kv_ag = dram.tile([4 * KVN], BF16)
        xf_in = dram.tile([E, TS], BF16)
        xf_ag = dram.tile([NC * E, TS], BF16, addr_space="Shared")
        kv_in_b = kv_in[:]
        kv_ag_b = kv_ag[:]

        def layernorm2(x_tiles, out_tiles):
            """Pure normalize (no affine) of two [128,768] f32 tiles."""
            stats = stat.tile([128, 2, 3, 6], F32, tag="bn_stats", name="bn_stats_t")
            for t in range(2):
                xr = x_tiles[t][:].rearrange("p (s d) -> p s d", s=3)
                for s in range(3):
                    nc.vector.bn_stats(out=stats[:, t, s, :], in_=xr[:, s, :])
            mv = stat.tile([128, 2, 2], F32, tag="bn_aggr", name="bn_aggr_t")
            for t in range(2):
                nc.vector.bn_aggr(out=mv[:, t, :], in_=stats[:, t])
            std = stat.tile([128, 2], F32, tag="std", name="std_t")
            for t in range(2):
                nc.scalar.activation(out=std[:, t:t + 1], in_=mv[:, t, 1:2],
                                     func=mybir.ActivationFunctionType.Sqrt,
                                     bias=eps_sb[:], scale=1.0)
            rstd = stat.tile([128, 2], F32, tag="rstd", name="rstd_t")
            nc.vector.reciprocal_approx_fast(out=rstd[:], in_=std[:])
            for t in range(2):
                nc.vector.tensor_scalar(out=out_tiles[t][:], in0=x_tiles[t][:],
                                        scalar1=mv[:, t, 0:1],
                                        scalar2=rstd[:, t:t + 1],
                                        op0=mybir.AluOpType.subtract,
                                        op1=mybir.AluOpType.mult)

        def scope(name):
            sid, _ = nc.enter_named_scope(name, False)
            _SCOPES.append((name, sid))

        def unscope():
            name, sid = _SCOPES.pop()
            nc.leave_named_scope(name, sid, False)

        _SCOPES = []

        es_l = es.enter_context(ExitStack())
        wqp = es_l.enter_context(tc.tile_pool(name="wqp", bufs=1))
        wkp = es_l.enter_context(tc.tile_pool(name="wkp", bufs=1))
        wvp = es_l.enter_context(tc.tile_pool(name="wvp", bufs=1))
        watpp = es_l.enter_context(tc.tile_pool(name="watpp", bufs=1))
        bias_p = es_l.enter_context(tc.tile_pool(name="bias_p", bufs=2))
        qTp = es_l.enter_context(tc.tile_pool(name="qTp", bufs=1))
        kvsp = es_l.enter_context(tc.tile_pool(name="kvsp", bufs=2))
        kTp = es_l.enter_context(tc.tile_pool(name="kTp", bufs=1))
        vp = es_l.enter_context(tc.tile_pool(name="vp", bufs=1))
        ep = es_l.enter_context(tc.tile_pool(name="ep", bufs=4))
        yp = es_l.enter_context(tc.tile_pool(name="yp", bufs=1))
        sm = es_l.enter_context(tc.tile_pool(name="sm", bufs=2))
        fcwp = es_l.enter_context(tc.tile_pool(name="fcwp", bufs=1))
        mTp = es_l.enter_context(tc.tile_pool(name="mTp", bufs=1))
        prwp = es_l.enter_context(tc.tile_pool(name="prwp", bufs=5))
        psT = es_l.enter_context(tc.tile_pool(name="psT", bufs=2, space="PSUM"))

        def transpose_to(hsrc, dst_tiles, dst_col):
            """hsrc [128,768] f32r -> bf16 dst_tiles[k][:, dst_col:dst_col+128]."""
            for k in range(6):
                pt = psT.tile([128, 128], F32R, tag="tr", name="tr")
                nc.tensor.transpose(pt[:], hsrc[:, k * 128:(k + 1) * 128], ident[:])
                dst = dst_tiles[k][:, dst_col:dst_col + 128]
                if k % 2 == 0:
                    nc.vector.tensor_copy(dst, pt[:].bitcast(F32))
                else:
                    nc.scalar.activation(out=dst, in_=pt[:].bitcast(F32),
                                         func=mybir.ActivationFunctionType.Copy)

        def load_kv_weights(layer):
            w = {}
            w["wk"] = [wkp.tile([128, E], BF16, tag=f"wk{k}", name=f"wk{k}")
                       for k in range(6)]
            for k in range(6):
                nc.sync.dma_start(out=w["wk"][k][:],
                                  in_=wk[layer, k * 128:(k + 1) * 128, :])
            w["bk"] = bias_p.tile([128, 6], F32, tag="bk", name="bk")
            nc.sync.dma_start(out=w["bk"][:], in_=bk[layer].transpose([1, 0]))
            w["wv"] = [wvp.tile([128, E], BF16, tag=f"wv{k}", name=f"wv{k}")
                       for k in range(6)]
            for k in range(6):
                nc.sync.dma_start(out=w["wv"][k][:],
                                  in_=wv[layer, k * 128:(k + 1) * 128, :])
            return w

        wcur = load_kv_weights(0)
        for j in range(8):
            nc.sync.dma_start(out=mask_sb[j][:], in_=mask[j])

        for layer in range(L):
            scope(f"L{layer}_qkv")
            wk_sb, wv_sb = wcur["wk"], wcur["wv"]
            bk_sb = wcur["bk"]
            # ---- LN1 (pure normalize) + transpose -> hT [768, 256] bf16 ----
            hT = [hTp.tile([128, TS], BF16, tag=f"hT{k}", name=f"hT{k}") for k in range(6)]
            h_t = [hpool.tile([128, E], F32R, tag="h", name="h") for t in range(2)]
            layernorm2(x_sb, h_t)
            for t in range(2):
                transpose_to(h_t[t], hT, t * 128)

            bv_bc = bcast_row(lnrow, bv[layer], 128, BF16, E, "bv")

            es_a = ExitStack()
            psK = es_a.enter_context(tc.tile_pool(name="psK", bufs=2, space="PSUM"))
            psV = es_a.enter_context(tc.tile_pool(name="psV", bufs=1, space="PSUM"))
            for m in range(6):
                ps = psK.tile([128, TS], F32, tag="kps", name="k_ps")
                for k in range(6):
                    nc.tensor.matmul(ps[:], wk_sb[k][:, m * 128:(m + 1) * 128],
                                     hT[k][:], start=(k == 0), stop=(k == 5))
                kt = kvsp.tile([128, TS], BF16, tag="kloc", name="kloc")
                nc.vector.tensor_scalar_add(out=kt[:], in0=ps[:],
                                            scalar1=bk_sb[:, m:m + 1])
                nc.sync.dma_start(
                    out=bass.AP(tensor=kv_in_b.tensor,
                                offset=kv_in_b.offset + m * 128 * TS,
                                ap=[[TS, 128], [1, TS]]),
                    in_=kt[:])

            # ---- V (token-major, ones column pre-packed) ----
            for t in range(2):
                ps = psV.tile([128, E], F32, tag="vps", name="v_ps")
                for n0, n1 in ((0, 512), (512, 768)):
                    for k in range(6):
                        nc.tensor.matmul(ps[:, n0:n1],
                                         hT[k][:, t * 128:(t + 1) * 128],
                                         wv_sb[k][:, n0:n1],
                                         start=(k == 0), stop=(k == 5))
                vt = kvsp.tile([128, H * 65], BF16, tag="vloc", name="vloc")
                nc.vector.tensor_copy(vt[:], ones780[:])
                for h in range(H):
                    nc.vector.tensor_add(out=vt[:, 65 * h:65 * h + 64],
                                         in0=ps[:, 64 * h:64 * h + 64],
                                         in1=bv_bc[:, 64 * h:64 * h + 64])
                nc.sync.dma_start(
                    out=bass.AP(tensor=kv_in_b.tensor,
                                offset=kv_in_b.offset + KB_EL + t * 128 * (H * 65),
                                ap=[[H * 65, 128], [1, H * 65]]),
                    in_=vt[:])
            nc.gpsimd.collective_compute(
                "AllGather", mybir.AluOpType.bypass,
                replica_groups=g_batch,
                ins=[kv_in[:]],
                outs=[kv_ag[:]],
            )
            if layer == 0:
                # warm the 8-rank ring for the final AllGather while L0 runs
                nc.gpsimd.collective_compute(
                    "AllGather", mybir.AluOpType.bypass, replica_groups=g_all,
                    ins=[warm_in[:]], outs=[warm_out2[:]])

            # ---- prefetch proj/MLP weights into the AllGather window ----
            watp_sb = [watpp.tile([64, E], BF16, tag=f"watp{h}", name=f"watp{h}")
                       for h in range(H)]
            for h in range(H):
                nc.sync.dma_start(out=watp_sb[h][:], in_=watp[layer, h])
            fck = [fcwp.tile([128, 4 * E], BF16, tag=f"fck{k}", name=f"fck{k}")
                   for k in range(6)]
            for k in range(6):
                nc.sync.dma_start(out=fck[k][:], in_=fcw[layer, k])
            fcb_sb = bias_p.tile([128, 24], F32, tag="fcb", name="fcb")
            nc.sync.dma_start(out=fcb_sb[:], in_=fcb[layer].transpose([1, 0]))

            # ---- Q per head (overlaps the AllGather) ----
            wq_sb = [wqp.tile([128, E], BF16, tag=f"wq{k}", name=f"wq{k}")
                     for k in range(6)]
            for k in range(6):
                nc.sync.dma_start(out=wq_sb[k][:],
                                  in_=wq[layer, k * 128:(k + 1) * 128, :])
            bq_sb = bias_p.tile([64, H], F32, tag="bq", name="bq")
            nc.sync.dma_start(out=bq_sb[:], in_=bq[layer].transpose([1, 0]))
            psQ = es_a.enter_context(tc.tile_pool(name="psQ", bufs=2, space="PSUM"))
            qT = [qTp.tile([64, TS], BF16, tag=f"qT{h}", name=f"qT{h}") for h in range(H)]
            for h in range(H):
                ps = psQ.tile([64, TS], F32, tag="q", name="q_ps")
                for k in range(6):
                    nc.tensor.matmul(ps[:], wq_sb[k][:, h * 64:(h + 1) * 64],
                                     hT[k][:], start=(k == 0), stop=(k == 5))
                nc.vector.tensor_scalar_add(out=qT[h][:], in0=ps[:],
                                            scalar1=bq_sb[:, h:h + 1])
            es_a.close()

            unscope()
            scope(f"L{layer}_attn")
            # ---- load gathered K (per-head) and V from the packed buffer ----
            kTh = [kTp.tile([64, T], BF16, tag=f"kTh{h}", name=f"kTh{h}")
                   for h in range(H)]
            v_sb = [vp.tile([128, H * 65], BF16, tag=f"v{j}", name=f"v{j}")
                    for j in range(8)]

            def load_kth(h):
                nc.sync.dma_start(
                    out=kTh[h][:].rearrange("p (r c) -> p r c", r=4),
                    in_=bass.AP(tensor=kv_ag_b.tensor,
                                offset=kv_ag_b.offset + h * 64 * TS,
                                ap=[[TS, 64], [KVN, 4], [1, TS]]))

            def load_v(j):
                nc.sync.dma_start(
                    out=v_sb[j][:],
                    in_=bass.AP(tensor=kv_ag_b.tensor,
                                offset=(kv_ag_b.offset + (j // 2) * KVN + KB_EL
                                        + (j % 2) * 128 * (H * 65)),
                                ap=[[H * 65, 128], [1, H * 65]]))

            load_kth(0)
            load_kth(1)
            for j in range(8):
                load_v(j)
            for h in range(2, H):
                load_kth(h)

            # prefetch next layer's K/V weights (behind the kv loads in priority)
            wnext = load_kv_weights(layer + 1) if layer + 1 < L else None

            # ---- attention: 12 heads, 256 queries vs 1024 masked keys ----
            es_b = ExitStack()
            psS = es_b.enter_context(tc.tile_pool(name="psS", bufs=2, space="PSUM"))
            psY = es_b.enter_context(tc.tile_pool(name="psY", bufs=2, space="PSUM"))
            yT2 = [yp.tile([64, 2 * TS], BF16, tag=f"yT{hp}", name=f"yT{hp}")
                   for hp in range(6)]

            def softmax_epilogue(hp, yps):
                # den rows -> approx recip -> PE broadcast -> mul
                den = sm.tile([1, 2 * TS], F32, tag="den", name="den")
                for i in range(2):
                    nc.vector.tensor_copy(den[:, i * TS:(i + 1) * TS],
                                          yps[i][64:65, :])
                rec = sm.tile([1, 2 * TS], F32, tag="rec", name="rec")
                nc.vector.reciprocal_approx_fast(out=rec[:], in_=den[:])
                rec_bf = sm.tile([1, 2 * TS], BF16, tag="rec_bf", name="rec_bf")
                nc.vector.tensor_copy(rec_bf[:], rec[:])
                psb = psT.tile([64, 2 * TS], F32, tag="tr", name="bc_ps")
                nc.tensor.matmul(psb[:], ones780[0:1, 0:64], rec_bf[:],
                                 start=True, stop=True)
                rbc = sm.tile([64, 2 * TS], BF16, tag="rbc", name="rbc")
                nc.vector.tensor_copy(rbc[:], psb[:])
                for i in range(2):
                    nc.vector.tensor_mul(out=yT2[hp][:, i * TS:(i + 1) * TS],
                                         in0=yps[i][0:64, :],
                                         in1=rbc[:, i * TS:(i + 1) * TS])

            # software-pipelined: scores for j+1 issue before AV for j, and
            # the previous head-pair's epilogue issues after this pair's
            # first scores, so the PE never waits on the exp/mask chain.
            pending = None
            for hp in range(6):
                yps = [psY.tile([65, TS], F32, tag=f"y{i}", name=f"y{i}")
                       for i in range(2)]
                e_hold = {}

                def av(j, last):
                    ej = e_hold.pop(j)
                    for i in range(2):
                        h = 2 * hp + i
                        nc.tensor.matmul(yps[i][:],
                                         v_sb[j][:, 65 * h:65 * h + 65],
                                         ej[:, i * TS:(i + 1) * TS],
                                         start=(j == 0), stop=last)

                for j in range(8):
                    pss = psS.tile([128, 2 * TS], F32, tag="s", name="s_ps")
                    e_sb = ep.tile([128, 2 * TS], BF16, tag="e", name="e")
                    for i in range(2):
                        nc.tensor.matmul(pss[:, i * TS:(i + 1) * TS],
                                         kTh[2 * hp + i][:, j * 128:(j + 1) * 128],
                                         qT[2 * hp + i][:], start=True, stop=True)
                    nc.scalar.activation(out=e_sb[:], in_=pss[:],
                                         func=mybir.ActivationFunctionType.Exp,
                                         scale=SCALE)
                    nc.vector.tensor_mul(out=e_sb[:], in0=e_sb[:], in1=mask_sb[j][:])
                    e_hold[j] = e_sb
                    if j == 2 and pending is not None:
                        softmax_epilogue(*pending)
                        pending = None
                    if j >= 2:
                        av(j - 2, False)
                av(6, False)
                av(7, True)
                pending = (hp, yps)
            softmax_epilogue(*pending)
            es_b.close()

            # ---- out-proj (token-local) + residual ----
            es_c = ExitStack()
            psO = es_c.enter_context(tc.tile_pool(name="psO", bufs=2, space="PSUM"))
            atpb_bc = bcast_row(lnrow, atpb[layer], 128, BF16, E, "atpb")
            for t in range(2):
                ps = psO.tile([128, E], F32, tag="o", name="o_ps")
                for n0, n1 in ((0, 512), (512, 768)):
                    for h in range(H):
                        nc.tensor.matmul(ps[:, n0:n1],
                                         yT2[h // 2][:, (h % 2) * TS + t * 128:
                                                     (h % 2) * TS + (t + 1) * 128],
                                         watp_sb[h][:, n0:n1],
                                         start=(h == 0), stop=(h == H - 1))
                nc.vector.tensor_add(out=x_sb[t][:], in0=x_sb[t][:], in1=ps[:])
                nc.vector.tensor_add(out=x_sb[t][:], in0=x_sb[t][:], in1=atpb_bc[:])
            es_c.close()

            unscope()
            scope(f"L{layer}_mlp")
            # ---- LN2 (pure normalize) + transpose ----
            h2T = [hTp.tile([128, TS], BF16, tag=f"hT{k}", name=f"hT{k}") for k in range(6)]
            h2_t = [hpool.tile([128, E], F32R, tag="h", name="h") for t in range(2)]
            layernorm2(x_sb, h2_t)
            for t in range(2):
                transpose_to(h2_t[t], h2T, t * 128)

            # ---- MLP fc: mT[m] = gelu(fc_w.T @ h2T + fc_b) ----
            es_d = ExitStack()
            psM = es_d.enter_context(tc.tile_pool(name="psM", bufs=2, space="PSUM"))
            psP = es_d.enter_context(tc.tile_pool(name="psP", bufs=1, space="PSUM"))
            mT = [mTp.tile([128, TS], BF16, tag=f"mT{m}", name=f"mT{m}") for m in range(24)]
            for m in range(24):
                ps = psM.tile([128, TS], F32, tag="m", name="m_ps")
                for k in range(6):
                    nc.tensor.matmul(ps[:], fck[k][:, m * 128:(m + 1) * 128],
                                     h2T[k][:], start=(k == 0), stop=(k == 5))
                nc.scalar.activation(out=mT[m][:], in_=ps[:],
                                     func=mybir.ActivationFunctionType.Gelu_apprx_tanh,
                                     bias=fcb_sb[:, m:m + 1])

            # ---- MLP pr + residual ----
            prb_bc = bcast_row(lnrow, prb[layer], 128, BF16, E, "prb")
            ps2 = [psP.tile([128, E], F32, tag=f"p{t}", name=f"p{t}") for t in range(2)]
            for kk in range(24):
                prw_sb = prwp.tile([128, E], BF16, tag="prw", name="prw")
                nc.sync.dma_start(out=prw_sb[:],
                                  in_=prw[layer, kk * 128:(kk + 1) * 128, :])
                for t in range(2):
                    for n0, n1 in ((0, 512), (512, 768)):
                        nc.tensor.matmul(ps2[t][:, n0:n1],
                                         mT[kk][:, t * 128:(t + 1) * 128],
                                         prw_sb[:, n0:n1],
                                         start=(kk == 0), stop=(kk == 23))
            for t in range(2):
                nc.vector.tensor_add(out=x_sb[t][:], in0=x_sb[t][:], in1=ps2[t][:])
                nc.vector.tensor_add(out=x_sb[t][:], in0=x_sb[t][:], in1=prb_bc[:])
            es_d.close()
            unscope()
            wcur = wnext

        # ---- final LN (affine folded into wte) + AllGather(all 8) ----
        scope("lnf_ag")
        xfT = [hTp.tile([128, TS], BF16, tag=f"hT{k}", name=f"hT{k}") for k in range(6)]
        xf_t = [hpool.tile([128, E], F32R, tag="h", name="h") for t in range(2)]
        layernorm2(x_sb, xf_t)
        for t in range(2):
            transpose_to(xf_t[t], xfT, t * 128)
        for k in range(6):
            nc.sync.dma_start(out=xf_in[k * 128:(k + 1) * 128, :], in_=xfT[k][:])
        es_l.close()

        es_h = es.enter_context(ExitStack())
        xfp = es_h.enter_context(tc.tile_pool(name="xfp", bufs=1))
        wtep = es_h.enter_context(tc.tile_pool(name="wtep", bufs=3))
        lop = es_h.enter_context(tc.tile_pool(name="lop", bufs=3))
        psL = es_h.enter_context(tc.tile_pool(name="psL", bufs=2, space="PSUM"))

        NG = 5                      # wte groups
        GW = VS // NG               # 1280 vocab cols per group
        NVG = GW // 128             # 10 vocab tiles per group

        def load_wte_group(g):
            tiles = [wtep.tile([128, GW], BF16, tag=f"wte{k}", name=f"wte{k}")
                     for k in range(6)]
            for k in range(6):
                nc.sync.dma_start(out=tiles[k][:],
                                  in_=wteT[k * 128:(k + 1) * 128, g * GW:(g + 1) * GW])
            return tiles

        wte_g0 = load_wte_group(0)      # prefetch during the AllGather

        nc.gpsimd.collective_compute(
            "AllGather", mybir.AluOpType.bypass,
            replica_groups=g_all,
            ins=[xf_in.opt()],
            outs=[xf_ag.opt()],
        )
        xf_ag_b = xf_ag[:]
        xf_sb = [xfp.tile([128, NC * TS], BF16, tag=f"xf{k}", name=f"xf{k}")
                 for k in range(6)]
        for k in range(6):
            nc.sync.dma_start(
                out=xf_sb[k][:].rearrange("p (r t) -> p r t", r=NC),
                in_=bass.AP(tensor=xf_ag_b.tensor,
                            offset=xf_ag_b.offset + k * 128 * TS,
                            ap=[[TS, 128], [E * TS, NC], [1, TS]]))
        unscope()

        scope("lmhead")
        for g in range(NG):
            wte_g = wte_g0 if g == 0 else load_wte_group(g)
            for v in range(NVG):
                ps = psL.tile([128, B * T], F32, tag="l", name="l_ps")
                for k in range(6):
                    stat_w = wte_g[k][:, v * 128:(v + 1) * 128]
                    for n in range(4):
                        nc.tensor.matmul(ps[:, n * 512:(n + 1) * 512],
                                         stat_w,
                                         xf_sb[k][:, n * 512:(n + 1) * 512],
                                         start=(k == 0), stop=(k == 5))
                lo = lop.tile([128, B * T], F32, tag="lo", name="lo")
                nc.vector.tensor_copy(lo[:, 0:1024], ps[:, 0:1024])
                nc.scalar.activation(out=lo[:, 1024:2048], in_=ps[:, 1024:2048],
                                     func=mybir.ActivationFunctionType.Copy)
                vg = g * NVG + v
                nc.sync.dma_start(out=logits[vg * 128:(vg + 1) * 128, :], in_=lo[:])
        unscope()

    nc.compile()
    return nc


def _prep_inputs(idx, wte, wpe, ln1_w, ln1_b, attn_w, attn_b, atp_w, atp_b,
                 ln2_w, ln2_b, fc_w, fc_b, pr_w, pr_b, lnf_w, lnf_b):
    idx = np.asarray(idx)
    f = lambda a: np.ascontiguousarray(np.asarray(a), dtype=np.float32)
    bf = lambda a: np.ascontiguousarray(np.asarray(a, dtype=np.float32).astype(BF))
    wte32, wpe32 = f(wte), f(wpe)
    x0 = wte32[idx.reshape(-1)] + np.tile(wpe32[:T], (B, 1))  # [2048, 768]

    ln1_w, ln1_b = f(ln1_w), f(ln1_b)
    ln2_w, ln2_b = f(ln2_w), f(ln2_b)
    lnf_w, lnf_b = f(lnf_w), f(lnf_b)

    # fold LN affine into the following matmuls (exact)
    aw = f(attn_w)
    attn_b = f(attn_b) + np.einsum("le,lef->lf", ln1_b, aw)
    attn_w = aw * ln1_w[:, :, None]
    fw = f(fc_w)
    fc_b2 = f(fc_b) + np.einsum("le,lef->lf", ln2_b, fw)
    fc_w2 = fw * ln2_w[:, :, None]
    wteT_scaled = (wte32 * lnf_w[None, :]).T  # [E, V]
    logit_bias = lnf_b @ wte32.T              # [V]

    wte_pad = np.zeros((E, VPAD), np.float32)
    wte_pad[:, :V] = wteT_scaled
    wteT_full = wte_pad.astype(BF)

    common = {
        "wq": bf(attn_w[:, :, 0:E]),
        "bq": np.ascontiguousarray(attn_b[:, 0:E].reshape(L, H, HD)),
        "wk": bf(attn_w[:, :, E:2 * E]),
        "bk": np.ascontiguousarray(attn_b[:, E:2 * E].reshape(L, 6, 128)),
        "wv": bf(attn_w[:, :, 2 * E:3 * E]),
        "bv": bf(attn_b[:, 2 * E:3 * E]),
        "watp": bf(np.asarray(atp_w).reshape(L, H, HD, E)),
        "atpb": bf(atp_b),
        "fcw": bf(fc_w2.reshape(L, 6, 128, 4 * E)),
        "fcb": np.ascontiguousarray(fc_b2.reshape(L, 24, 128)),
        "prw": bf(pr_w), "prb": bf(pr_b),
    }
    in_maps = []
    kidx = np.arange(128)
    qidx = np.arange(TS)
    for c in range(NC):
        r = c % 4
        m = np.zeros((8, 128, TS), np.float32)
        for j in range(8):
            m[j] = ((128 * j + kidx)[:, None] <= (TS * r + qidx)[None, :])
        m2 = np.concatenate([m, m], axis=2)
        in_maps.append({
            **common,
            "x0s": np.ascontiguousarray(x0[c * TS:(c + 1) * TS]),
            "mask": m2.astype(BF),
            "wteT": np.ascontiguousarray(wteT_full[:, c * VS:(c + 1) * VS]),
        })
    return in_maps, logit_bias


def kernel(trace=False, **inputs):
    if "nc" not in _CACHE:
        _CACHE["nc"] = _build_program()
    nc = _CACHE["nc"]
    in_maps, logit_bias = _prep_inputs(**inputs)
    res = run_bass_kernel_spmd(nc, in_maps, core_ids=list(range(NC)), trace=trace)
    _CACHE["last_result"] = res
    logits = np.concatenate([res.results[c]["logits"] for c in range(NC)], axis=0)
    out = np.ascontiguousarray(logits[:V].T)  # [B*T, V]
    if np.any(logit_bias):
        out += logit_bias[None, :]
    return out.reshape(B, T, V).astype(np.float32)
